# revision 4
# baseline (speedup 1.0000x reference)
"""MatchSAGE (2-layer GraphSAGE x2 encoders + edge predictors) on 8 trn2
NeuronCores via Bass/Tile.

Strategy (graph/data parallel, per the sharding hint):
  - Nodes are globally sorted by in-degree (desc) and dealt round-robin to
    the 8 cores -> per-core slot order has descending degree and near-equal
    edge counts, so one SPMD program (static control flow) fits all cores.
  - segment-mean aggregation runs as "rounds": in round r every live slot
    (node with deg > r) gathers its r-th in-neighbor's feature row via
    indirect DMA with compute_op=add, accumulating IN SBUF. Pad slots point
    at a dedicated zero row. No one-hot matmuls, no scatter.
  - After each layer the per-core node shard is AllGathered so every core
    holds the full h table (needed for next layer's gathers / predictors).
  - Edge predictors (pos/neg dots, MLP+BN rating) shard edges contiguously
    across cores; BatchNorm stats go through a tiny AllReduce.

All tensors keep float32. Host-side work is limited to index remapping /
layout (degree sort, CSR, padding, transposes of inputs).
"""
import sys
sys.path.insert(0, "/opt/trn_rl_repo")
import numpy as np

import concourse.bass as bass
import concourse.mybir as mybir
import concourse.tile as tile
from concourse import bacc
from concourse.bass import ts
from concourse.masks import make_identity

P = 128
f32 = mybir.dt.float32
i32 = mybir.dt.int32
AF = mybir.ActivationFunctionType
ALU = mybir.AluOpType
EPS = 1e-5


class Cfg:
    def __init__(self, N, E, EP, EN, C=8):
        self.N, self.E, self.EP, self.EN, self.C = N, E, EP, EN, C
        self.SLOTS = ((N + C - 1) // C + P - 1) // P * P   # per-core slots
        self.NS = self.SLOTS // P                           # node subtiles
        self.NROW = C * self.SLOTS + 1                      # table rows
        self.ZROW = C * self.SLOTS                          # zero row index
        assert EP % C == 0 and EN % C == 0 and E % C == 0
        self.EPC, self.ENC, self.EC = EP // C, EN // C, E // C
        self.PSUB = (self.EPC + P - 1) // P
        self.NSUB = (self.ENC + P - 1) // P
        self.RSUB = self.EC // P
        assert self.EC % P == 0, "rating edge chunk must be 128-divisible"


FULL = Cfg(N=100000, E=640000, EP=500000, EN=500000)


# --------------------------------------------------------------------------
# host-side preprocessing
# --------------------------------------------------------------------------

def prep_inputs(cfg, x1, x2, g_src, g_dst, pos_src, pos_dst, neg_src,
                neg_dst):
    C, N, E = cfg.C, cfg.N, cfg.E
    g_src = np.asarray(g_src).astype(np.int64)
    g_dst = np.asarray(g_dst).astype(np.int64)
    deg = np.bincount(g_dst, minlength=N)
    order = np.argsort(-deg, kind="stable")
    rank = np.empty(N, np.int64)
    rank[order] = np.arange(N)
    row_of = (rank % C) * cfg.SLOTS + rank // C      # node -> table row

    # CSR by dst
    e_order = np.argsort(g_dst, kind="stable")
    srcs_by_dst = g_src[e_order]
    starts = np.zeros(N + 1, np.int64)
    starts[1:] = np.cumsum(deg)

    maxdeg = int(deg.max())
    deg_sorted = deg[order]                           # descending
    cnt_global = np.searchsorted(-deg_sorted, -np.arange(1, maxdeg + 1),
                                 side="right")
    # cnt_global[r] = #nodes with deg > r  (r-th edge exists iff deg > r)
    subtiles_r = [int(np.ceil(np.ceil(cnt_global[r] / C) / P))
                  for r in range(maxdeg)]
    subtiles_r[0] = cfg.NS    # round 0 covers ALL slots (zero-fill bypass)
    G = sum(subtiles_r)

    gidx = np.full((C, P, G), cfg.ZROW, np.int32)
    col = 0
    for r in range(maxdeg):
        S = subtiles_r[r]
        slots = np.arange(S * P)
        for c in range(C):
            ranks = slots * C + c
            valid = ranks < N
            nodes = order[np.minimum(ranks, N - 1)]
            has = valid & (deg[nodes] > r)
            colvals = np.full(S * P, cfg.ZROW, np.int64)
            colvals[has] = row_of[srcs_by_dst[starts[nodes[has]] + r]]
            gidx[c, :, col:col + S] = colvals.reshape(S, P).T
        col += S

    # per-slot 1/max(deg,1) strip [C, P, NS]
    slots = np.arange(cfg.SLOTS)
    invd = np.ones((C, P, cfg.NS), np.float32)
    xT = np.zeros((2, C, P, cfg.SLOTS), np.float32)
    xfull = np.zeros((2, cfg.NROW, P), np.float32)
    xfull[0, row_of] = np.asarray(x1, np.float32)
    xfull[1, row_of] = np.asarray(x2, np.float32)
    for c in range(C):
        ranks = slots * C + c
        valid = ranks < N
        nodes = order[np.minimum(ranks, N - 1)]
        iv = np.ones(cfg.SLOTS, np.float32)
        iv[valid] = 1.0 / np.maximum(deg[nodes[valid]], 1)
        invd[c] = iv.reshape(cfg.NS, P).T
        xs = np.zeros((cfg.SLOTS, P), np.float32)
        xs[valid] = np.asarray(x1, np.float32)[nodes[valid]]
        xT[0, c] = xs.T
        xs = np.zeros((cfg.SLOTS, P), np.float32)
        xs[valid] = np.asarray(x2, np.float32)[nodes[valid]]
        xT[1, c] = xs.T

    def edge_strip(idx_arr, nsub, chunk):
        out = np.full((C, P, nsub), cfg.ZROW, np.int32)
        a = row_of[np.asarray(idx_arr).astype(np.int64)]
        for c in range(C):
            ch = a[c * chunk:(c + 1) * chunk]
            pad = np.full(nsub * P, cfg.ZROW, np.int64)
            pad[:len(ch)] = ch
            out[c] = pad.reshape(nsub, P).T
        return out

    strips = dict(
        pos_s=edge_strip(pos_src, cfg.PSUB, cfg.EPC),
        pos_d=edge_strip(pos_dst, cfg.PSUB, cfg.EPC),
        neg_s=edge_strip(neg_src, cfg.NSUB, cfg.ENC),
        neg_d=edge_strip(neg_dst, cfg.NSUB, cfg.ENC),
        g_s=edge_strip(g_src, cfg.RSUB, cfg.EC),
        g_d=edge_strip(g_dst, cfg.RSUB, cfg.EC),
    )
    return dict(gidx=gidx, invd=invd, xT=xT, xfull=xfull,
                subtiles_r=subtiles_r, strips=strips)


# --------------------------------------------------------------------------
# device kernel
# --------------------------------------------------------------------------

def build_kernel(cfg, subtiles_r):
    C, NS, NROW, SLOTS = cfg.C, cfg.NS, cfg.NROW, cfg.SLOTS
    G = sum(subtiles_r)
    nc = bacc.Bacc(None, target_bir_lowering=False, debug=False)

    xs_full = nc.dram_tensor("xs_full", [NROW, P], f32, kind="ExternalInput")
    xd_full = nc.dram_tensor("xd_full", [NROW, P], f32, kind="ExternalInput")
    xsT = nc.dram_tensor("xsT", [P, SLOTS], f32, kind="ExternalInput")
    xdT = nc.dram_tensor("xdT", [P, SLOTS], f32, kind="ExternalInput")
    gidx_d = nc.dram_tensor("gidx", [P, G], i32, kind="ExternalInput")
    invd_d = nc.dram_tensor("invd", [P, NS], f32, kind="ExternalInput")
    pos_s_d = nc.dram_tensor("pos_s", [P, cfg.PSUB], i32, kind="ExternalInput")
    pos_d_d = nc.dram_tensor("pos_d", [P, cfg.PSUB], i32, kind="ExternalInput")
    neg_s_d = nc.dram_tensor("neg_s", [P, cfg.NSUB], i32, kind="ExternalInput")
    neg_d_d = nc.dram_tensor("neg_d", [P, cfg.NSUB], i32, kind="ExternalInput")
    g_s_d = nc.dram_tensor("g_s", [P, cfg.RSUB], i32, kind="ExternalInput")
    g_d_d = nc.dram_tensor("g_d", [P, cfg.RSUB], i32, kind="ExternalInput")
    Ws1_d = nc.dram_tensor("Ws1", [P, P], f32, kind="ExternalInput")
    Wn1_d = nc.dram_tensor("Wn1", [P, P], f32, kind="ExternalInput")
    Ws2_d = nc.dram_tensor("Ws2", [P, P], f32, kind="ExternalInput")
    Wn2_d = nc.dram_tensor("Wn2", [P, P], f32, kind="ExternalInput")
    b1_d = nc.dram_tensor("b1r", [1, P], f32, kind="ExternalInput")
    b2_d = nc.dram_tensor("b2r", [1, P], f32, kind="ExternalInput")
    W1m_d = nc.dram_tensor("W1m", [P, 30], f32, kind="ExternalInput")
    W2m_d = nc.dram_tensor("W2m", [30, 1], f32, kind="ExternalInput")
    b1m_d = nc.dram_tensor("b1m", [30, 1], f32, kind="ExternalInput")
    gam_d = nc.dram_tensor("gam", [30, 1], f32, kind="ExternalInput")
    bet_d = nc.dram_tensor("bet", [30, 1], f32, kind="ExternalInput")
    b2m_d = nc.dram_tensor("b2m", [1, 1], f32, kind="ExternalInput")

    pos_o = nc.dram_tensor("pos_o", [cfg.PSUB * P, 1], f32,
                           kind="ExternalOutput")
    neg_o = nc.dram_tensor("neg_o", [cfg.NSUB * P, 1], f32,
                           kind="ExternalOutput")
    rat_o = nc.dram_tensor("rat_o", [cfg.RSUB * P, 1], f32,
                           kind="ExternalOutput")

    with tile.TileContext(nc) as tc:
        with tc.tile_pool(name="cpool", bufs=1) as cpool, \
             tc.tile_pool(name="accp", bufs=2) as accp, \
             tc.tile_pool(name="wp", bufs=4) as wp, \
             tc.tile_pool(name="uvp", bufs=8) as uvp, \
             tc.tile_pool(name="pp", bufs=2, space="PSUM") as pp, \
             tc.tile_pool(name="pp1", bufs=2, space="PSUM") as pp1, \
             tc.tile_pool(name="dram", bufs=1, space="DRAM") as dram:

            # ---------- constants / strips ----------
            def load_const(d, shape, dtype=f32):
                t = cpool.tile(shape, dtype, tag=d.tensor.name
                               if hasattr(d, "tensor") else d.name)
                nc.sync.dma_start(out=t[:], in_=d[:, :])
                return t

            gidx_sb = load_const(gidx_d, [P, G], i32)
            invd_sb = load_const(invd_d, [P, NS])
            pos_s_sb = load_const(pos_s_d, [P, cfg.PSUB], i32)
            pos_d_sb = load_const(pos_d_d, [P, cfg.PSUB], i32)
            neg_s_sb = load_const(neg_s_d, [P, cfg.NSUB], i32)
            neg_d_sb = load_const(neg_d_d, [P, cfg.NSUB], i32)
            g_s_sb = load_const(g_s_d, [P, cfg.RSUB], i32)
            g_d_sb = load_const(g_d_d, [P, cfg.RSUB], i32)
            Ws1 = load_const(Ws1_d, [P, P])
            Wn1 = load_const(Wn1_d, [P, P])
            Ws2 = load_const(Ws2_d, [P, P])
            Wn2 = load_const(Wn2_d, [P, P])
            b1r = load_const(b1_d, [1, P])
            b2r = load_const(b2_d, [1, P])
            W1m = load_const(W1m_d, [P, 30])
            W2m = load_const(W2m_d, [30, 1])
            b1m = load_const(b1m_d, [30, 1])
            gam = load_const(gam_d, [30, 1])
            bet = load_const(bet_d, [30, 1])
            b2m = load_const(b2m_d, [1, 1])

            ident = cpool.tile([P, P], f32)
            make_identity(nc, ident[:])
            ones_row = cpool.tile([1, P], f32)
            nc.vector.memset(ones_row[:], 1.0)
            zrow_sb = cpool.tile([1, P], f32)
            nc.vector.memset(zrow_sb[:], 0.0)

            # ---------- DRAM intermediates ----------
            h1s_sh = dram.tile([SLOTS, P], f32)
            h1d_sh = dram.tile([SLOTS, P], f32)
            h2s_sh = dram.tile([SLOTS, P], f32)
            h2d_sh = dram.tile([SLOTS, P], f32)
            h1s_f = dram.tile([NROW, P], f32)
            h1d_f = dram.tile([NROW, P], f32)
            h2s_f = dram.tile([NROW, P], f32)
            h2d_f = dram.tile([NROW, P], f32)
            zscr = dram.tile([30, cfg.RSUB * P], f32)
            cc_in = dram.tile([30, 2], f32)
            cc_out = dram.tile([30, 2], f32)

            def gather(dst_ap, table, idx_col, accumulate):
                nc.gpsimd.indirect_dma_start(
                    out=dst_ap, out_offset=None, in_=table[:, :],
                    in_offset=bass.IndirectOffsetOnAxis(ap=idx_col, axis=0),
                    compute_op=ALU.add if accumulate else ALU.bypass)

            def spmm(table, xT_dram, self_shard, Ws, Wn, brow, relu,
                     out_shard, name):
                """One SAGE layer for one encoder on this core's nodes."""
                acc = accp.tile([P, NS * P], f32, tag="acc")
                col = 0
                for r, S in enumerate(subtiles_r):
                    for t in range(S):
                        gather(acc[:, ts(t, P)], table,
                               gidx_sb[:, col:col + 1], r > 0)
                        col += 1
                for t in range(NS):
                    m_sb = wp.tile([P, P], f32, tag="m")
                    nc.vector.tensor_scalar_mul(
                        out=m_sb[:], in0=acc[:, ts(t, P)],
                        scalar1=invd_sb[:, t:t + 1])
                    trp = pp.tile([P, P], f32, tag="trps")
                    nc.tensor.transpose(out=trp[:], in_=m_sb[:],
                                        identity=ident[:])
                    meanT = wp.tile([P, P], f32, tag="mt")
                    nc.vector.tensor_copy(out=meanT[:], in_=trp[:])
                    lh = wp.tile([P, P], f32, tag="lh")
                    if xT_dram is not None:
                        nc.sync.dma_start(out=lh[:], in_=xT_dram[:, ts(t, P)])
                    else:
                        hrow = wp.tile([P, P], f32, tag="hr")
                        nc.sync.dma_start(out=hrow[:],
                                          in_=self_shard[ts(t, P), :])
                        trp2 = pp.tile([P, P], f32, tag="trps")
                        nc.tensor.transpose(out=trp2[:], in_=hrow[:],
                                            identity=ident[:])
                        nc.vector.tensor_copy(out=lh[:], in_=trp2[:])
                    hp = pp1.tile([P, P], f32, tag="hps")
                    nc.tensor.matmul(out=hp[:], lhsT=lh[:], rhs=Ws[:],
                                     start=True, stop=False)
                    nc.tensor.matmul(out=hp[:], lhsT=meanT[:], rhs=Wn[:],
                                     start=False, stop=False)
                    nc.tensor.matmul(out=hp[:], lhsT=ones_row[:1, :],
                                     rhs=brow[:1, :], start=False, stop=True)
                    h_sb = wp.tile([P, P], f32, tag="h")
                    nc.scalar.activation(out=h_sb[:], in_=hp[:],
                                         func=AF.Relu if relu else AF.Copy)
                    nc.sync.dma_start(out=out_shard[ts(t, P), :], in_=h_sb[:])

            def allgather(shard, full):
                nc.gpsimd.collective_compute(
                    "AllGather", ALU.bypass,
                    replica_groups=[list(range(C))],
                    ins=[shard[:].opt()],
                    outs=[full[0:C * SLOTS, :].opt()])
                nc.sync.dma_start(out=full[C * SLOTS:C * SLOTS + 1, :],
                                  in_=zrow_sb[:1, :])

            # ---------- encoders ----------
            spmm(xs_full, xsT, None, Ws1, Wn1, b1r, True, h1s_sh, "e1l1")
            spmm(xd_full, xdT, None, Ws1, Wn1, b1r, True, h1d_sh, "e2l1")
            allgather(h1s_sh, h1s_f)
            allgather(h1d_sh, h1d_f)
            spmm(h1s_f, None, h1s_sh, Ws2, Wn2, b2r, False, h2s_sh, "e1l2")
            spmm(h1d_f, None, h1d_sh, Ws2, Wn2, b2r, False, h2d_sh, "e2l2")
            allgather(h2s_sh, h2s_f)
            allgather(h2d_sh, h2d_f)

            # ---------- rating pass 1 (z + BN stats) ----------
            szs = cpool.tile([30, cfg.RSUB], f32)
            sqs = cpool.tile([30, cfg.RSUB], f32)
            for s in range(cfg.RSUB):
                u = uvp.tile([P, P], f32, tag="u")
                gather(u[:], h2s_f, g_s_sb[:, s:s + 1], False)
                v = uvp.tile([P, P], f32, tag="v")
                gather(v[:], h2d_f, g_d_sb[:, s:s + 1], False)
                uv = wp.tile([P, P], f32, tag="uv")
                nc.vector.tensor_tensor(out=uv[:], in0=u[:], in1=v[:],
                                        op=ALU.mult)
                trp = pp.tile([P, P], f32, tag="trps")
                nc.tensor.transpose(out=trp[:], in_=uv[:], identity=ident[:])
                eT = wp.tile([P, P], f32, tag="et")
                nc.vector.tensor_copy(out=eT[:], in_=trp[:])
                zp = pp1.tile([30, P], f32, tag="zps")
                nc.tensor.matmul(out=zp[:], lhsT=W1m[:, :30], rhs=eT[:],
                                 start=True, stop=True)
                zT = wp.tile([30, P], f32, tag="zt")
                nc.scalar.activation(out=zT[:], in_=zp[:], func=AF.Identity,
                                     bias=b1m[:, :1],
                                     accum_out=szs[:, s:s + 1])
                zsq = wp.tile([30, P], f32, tag="zq")
                nc.scalar.activation(out=zsq[:], in_=zp[:], func=AF.Square,
                                     bias=b1m[:, :1],
                                     accum_out=sqs[:, s:s + 1])
                nc.sync.dma_start(out=zscr[:, ts(s, P)], in_=zT[:])

            sz_t = cpool.tile([30, 2], f32)
            nc.vector.tensor_reduce(out=sz_t[:, 0:1], in_=szs[:],
                                    axis=mybir.AxisListType.X, op=ALU.add)
            nc.vector.tensor_reduce(out=sz_t[:, 1:2], in_=sqs[:],
                                    axis=mybir.AxisListType.X, op=ALU.add)
            nc.sync.dma_start(out=cc_in[:, :], in_=sz_t[:])
            nc.gpsimd.collective_compute(
                "AllReduce", ALU.add, replica_groups=[list(range(C))],
                ins=[cc_in[:].opt()], outs=[cc_out[:].opt()])

            # ---------- pos / neg dots (overlap the AllReduce) ----------
            def dots(s_sb, d_sb, nsub, out_d):
                for s in range(nsub):
                    u = uvp.tile([P, P], f32, tag="u")
                    gather(u[:], h2s_f, s_sb[:, s:s + 1], False)
                    v = uvp.tile([P, P], f32, tag="v")
                    gather(v[:], h2d_f, d_sb[:, s:s + 1], False)
                    uv = wp.tile([P, P], f32, tag="uv")
                    nc.vector.tensor_tensor(out=uv[:], in0=u[:], in1=v[:],
                                            op=ALU.mult)
                    d = wp.tile([P, 1], f32, tag="d")
                    nc.vector.tensor_reduce(out=d[:], in_=uv[:],
                                            axis=mybir.AxisListType.X,
                                            op=ALU.add)
                    nc.sync.dma_start(out=out_d[ts(s, P), :], in_=d[:])

            dots(pos_s_sb, pos_d_sb, cfg.PSUB, pos_o)
            dots(neg_s_sb, neg_d_sb, cfg.NSUB, neg_o)

            # ---------- BN affine from global stats ----------
            st = cpool.tile([30, 2], f32)
            nc.sync.dma_start(out=st[:], in_=cc_out[:, :])
            inv_e = 1.0 / cfg.E
            mu = cpool.tile([30, 1], f32)
            nc.vector.tensor_scalar_mul(out=mu[:], in0=st[:, 0:1],
                                        scalar1=inv_e)
            ex2 = cpool.tile([30, 1], f32)
            nc.vector.tensor_scalar_mul(out=ex2[:], in0=st[:, 1:2],
                                        scalar1=inv_e)
            var = cpool.tile([30, 1], f32)
            nc.vector.tensor_tensor(out=var[:], in0=mu[:], in1=mu[:],
                                    op=ALU.mult)
            nc.vector.tensor_tensor(out=var[:], in0=ex2[:], in1=var[:],
                                    op=ALU.subtract)
            nc.vector.tensor_scalar_add(out=var[:], in0=var[:], scalar1=EPS)
            std = cpool.tile([30, 1], f32)
            nc.scalar.activation(out=std[:], in_=var[:], func=AF.Sqrt)
            rstd = cpool.tile([30, 1], f32)
            nc.vector.reciprocal(out=rstd[:], in_=std[:])
            scl = cpool.tile([30, 1], f32)
            nc.vector.tensor_tensor(out=scl[:], in0=gam[:], in1=rstd[:],
                                    op=ALU.mult)
            shf = cpool.tile([30, 1], f32)
            nc.vector.tensor_tensor(out=shf[:], in0=mu[:], in1=scl[:],
                                    op=ALU.mult)
            nc.vector.tensor_tensor(out=shf[:], in0=bet[:], in1=shf[:],
                                    op=ALU.subtract)

            # ---------- rating pass 2 ----------
            for s in range(cfg.RSUB):
                zin = wp.tile([30, P], f32, tag="zi")
                nc.sync.dma_start(out=zin[:], in_=zscr[:, ts(s, P)])
                rl = wp.tile([30, P], f32, tag="rl")
                nc.scalar.activation(out=rl[:], in_=zin[:], func=AF.Relu,
                                     bias=shf[:, :1], scale=scl[:, :1])
                rp = pp1.tile([1, P], f32, tag="rps")
                nc.tensor.matmul(out=rp[:], lhsT=W2m[:30, :1], rhs=rl[:],
                                 start=True, stop=True)
                r_sb = wp.tile([1, P], f32, tag="r")
                nc.scalar.activation(out=r_sb[:], in_=rp[:], func=AF.Identity,
                                     bias=b2m[:1, :1])
                nc.sync.dma_start(out=rat_o[ts(s, P), :], in_=r_sb[:1, :])

    nc.finalize()
    return nc


# --------------------------------------------------------------------------
# public entry point
# --------------------------------------------------------------------------

def make_in_maps(cfg, prep, weights):
    (Ws1, Wn1, b1, Ws2, Wn2, b2, W1m, b1m, gamma, beta, W2m, b2m) = weights
    f = np.float32
    common = dict(
        Ws1=np.asarray(Ws1, f), Wn1=np.asarray(Wn1, f),
        Ws2=np.asarray(Ws2, f), Wn2=np.asarray(Wn2, f),
        b1r=np.asarray(b1, f).reshape(1, P),
        b2r=np.asarray(b2, f).reshape(1, P),
        W1m=np.asarray(W1m, f), W2m=np.asarray(W2m, f),
        b1m=np.asarray(b1m, f).reshape(30, 1),
        gam=np.asarray(gamma, f).reshape(30, 1),
        bet=np.asarray(beta, f).reshape(30, 1),
        b2m=np.asarray(b2m, f).reshape(1, 1),
    )
    st = prep["strips"]
    maps = []
    for c in range(cfg.C):
        m = dict(common)
        m.update(
            xs_full=prep["xfull"][0], xd_full=prep["xfull"][1],
            xsT=np.ascontiguousarray(prep["xT"][0, c]),
            xdT=np.ascontiguousarray(prep["xT"][1, c]),
            gidx=np.ascontiguousarray(prep["gidx"][c]),
            invd=np.ascontiguousarray(prep["invd"][c]),
            pos_s=np.ascontiguousarray(st["pos_s"][c]),
            pos_d=np.ascontiguousarray(st["pos_d"][c]),
            neg_s=np.ascontiguousarray(st["neg_s"][c]),
            neg_d=np.ascontiguousarray(st["neg_d"][c]),
            g_s=np.ascontiguousarray(st["g_s"][c]),
            g_d=np.ascontiguousarray(st["g_d"][c]),
        )
        maps.append(m)
    return maps


def run(cfg, inputs, timeit=False):
    """Build, run on 8 cores, assemble full outputs."""
    from runner import SpmdRunner
    prep = prep_inputs(cfg, inputs["x1"], inputs["x2"], inputs["g_src"],
                       inputs["g_dst"], inputs["pos_src"], inputs["pos_dst"],
                       inputs["neg_src"], inputs["neg_dst"])
    weights = (inputs["Ws1"], inputs["Wn1"], inputs["b1"], inputs["Ws2"],
               inputs["Wn2"], inputs["b2"], inputs["W1m"], inputs["b1m"],
               inputs["gamma"], inputs["beta"], inputs["W2m"], inputs["b2m"])
    nc = build_kernel(cfg, prep["subtiles_r"])
    in_maps = make_in_maps(cfg, prep, weights)
    runner = SpmdRunner(nc, cfg.C)
    runner.set_inputs(in_maps)
    results, dt = runner.run()
    info = {"first_s": dt}
    if timeit:
        best, all_ts = runner.time_steady(warmup=1, iters=6)
        info["steady_s"] = best
        info["all_s"] = all_ts

    pos = np.concatenate([results[c]["pos_o"][:cfg.EPC] for c in range(cfg.C)])
    neg = np.concatenate([results[c]["neg_o"][:cfg.ENC] for c in range(cfg.C)])
    rat = np.concatenate([results[c]["rat_o"][:cfg.EC] for c in range(cfg.C)])
    return (pos.astype(np.float32), neg.astype(np.float32),
            rat.astype(np.float32)), info


def kernel(**inputs):
    out, _ = run(FULL, inputs)
    return out
